# revision 7
# baseline (speedup 1.0000x reference)
"""Causal multi-head attention (B=2, T=2048, C=1024, H=16, d=64) on 8 trn2 cores.

Sharding: core i -> (batch b = i//4, head group g = i%4, 4 heads/core).
Data parallel over B, tensor parallel over heads; the out-proj partial sums
(contraction over this core's 256 channels) are reduced on the host during
the gather step, along with b_proj and the analytically-folded V bias.

Device kernel works entirely in [feature, token] (transposed) layout so no
on-device transposes are needed:
  stage 1: Q^T,K^T = (Wqk)^T x^T   (+bias, 1/sqrt(d) folded into Wq host-side)
           V       = x W_v         (natural layout, used as stage-4 lhsT)
  stage 2: S^T[j,q] = K_h^T.T @ Q_h^T   per head, causal tiles only
  stage 3: P^T = exp(S^T + mask)        (no max subtraction; scores are O(20))
  stage 4: outT[65,q] = [V_h | 1]^T.T @ P^T  accumulated over j tiles
           (row 64 = softmax denominator Z)
  stage 5: att^T = outT[0:64] * (1/Z broadcast via PE rank-1)
  stage 6: y^T = Wp.T @ att^T  -> DMA out; host sums partials + transposes.
"""

import numpy as np

import concourse.bass as bass
import concourse.mybir as mybir
from concourse import bacc
import concourse.tile as tile
from concourse.bass_utils import run_bass_kernel_spmd

B, T, C, H, D = 2, 2048, 1024, 16, 64
NCORES = 8
HPC = 4            # heads per core
CS = HPC * D       # 256 channels per core (per Q/K/V block)
KT = C // 128      # 8 contraction tiles for the projections
NT = T // 128      # 16 token tiles of 128
QB = 512           # query block (psum bank width in fp32)
NQB = T // QB      # 4 query blocks
NEG = -1e9

F32 = mybir.dt.float32
F32R = mybir.dt.float32r  # PE full rate at N>=256; matmul-input dtype

TRACE = False
LAST_RESULT = None


def _build_body(nc, tc, ctx, xT, wqk, wv, bqk, wp, masks, ones4, ones64, yT):
    AF = mybir.ActivationFunctionType

    persist = ctx.enter_context(tc.tile_pool(name="persist", bufs=1))

    wqk_sb = [persist.tile([128, 2 * CS], F32R, tag=f"wqk{k}", name=f"wqk{k}") for k in range(KT)]
    wv_sb = [persist.tile([128, CS], F32R, tag=f"wv{k}", name=f"wv{k}") for k in range(KT)]
    bqk_sb = [persist.tile([128, 1], F32, tag=f"bqk{c}", name=f"bqk{c}") for c in range(4)]
    wp_sb = [persist.tile([128, C], F32R, tag=f"wp{k}", name=f"wp{k}") for k in range(2)]
    mask_sb = persist.tile([128, 4, QB], F32, tag="mask", name="mask_sb")
    qT_sb = [persist.tile([128, T], F32R, tag=f"qT{i}", name=f"qT{i}") for i in range(2)]
    kT_sb = [persist.tile([128, T], F32R, tag=f"kT{i}", name=f"kT{i}") for i in range(2)]
    v_sb = [persist.tile([128, HPC, D + 1], F32R, tag=f"v{t}", name=f"v{t}") for t in range(NT)]
    attT_sb = [persist.tile([128, T], F32R, tag=f"attT{i}", name=f"attT{i}") for i in range(2)]
    ones_sb = persist.tile([1, D], F32R, tag="ones", name="ones_sb")

    for k in range(KT):
        nc.sync.dma_start(out=wqk_sb[k][:, :], in_=wqk[k * 128:(k + 1) * 128, :])
        nc.sync.dma_start(out=wv_sb[k][:, :], in_=wv[k * 128:(k + 1) * 128, :])
    for c4 in range(4):
        nc.sync.dma_start(out=bqk_sb[c4][:, :], in_=bqk[c4 * 128:(c4 + 1) * 128, :])
    for k in range(2):
        nc.sync.dma_start(out=wp_sb[k][:, :], in_=wp[k * 128:(k + 1) * 128, :])
    nc.sync.dma_start(out=mask_sb[:, :, :], in_=masks[:, :, :])
    nc.sync.dma_start(out=ones_sb[:, :], in_=ones64[:, :])
    for t in range(NT):
        nc.sync.dma_start(out=v_sb[t][:, :, D], in_=ones4[:, :])

    # ---------------- stage 1: projections ----------------
    with (
        tc.tile_pool(name="xpool", bufs=1) as xpool,
        tc.tile_pool(name="s1psum", bufs=3, space="PSUM") as s1p,
    ):
        xT_sb = [xpool.tile([128, T], F32R, tag=f"xT{k}", name=f"xT{k}") for k in range(KT)]
        for k in range(KT):
            nc.sync.dma_start(out=xT_sb[k][:, :], in_=xT[k * 128:(k + 1) * 128, :])

        # Q^T (c-tiles 0,1) and K^T (c-tiles 2,3): out[c, t] over k
        for ct in range(4):
            dst = qT_sb[ct] if ct < 2 else kT_sb[ct - 2]
            for tc4 in range(NQB):
                ps = s1p.tile([128, QB], F32, tag="ps", name="ps")
                for k in range(KT):
                    nc.tensor.matmul(
                        ps[:, :],
                        lhsT=wqk_sb[k][:, ct * 128:(ct + 1) * 128],
                        rhs=xT_sb[k][:, tc4 * QB:(tc4 + 1) * QB],
                        start=(k == 0),
                        stop=(k == KT - 1),
                    )
                nc.vector.tensor_scalar_add(
                    dst[:, tc4 * QB:(tc4 + 1) * QB], ps[:, :], bqk_sb[ct][:, :]
                )

        # V natural layout: V[t, c] over k; ones column appended per head
        for t in range(NT):
            ps = s1p.tile([128, QB], F32, tag="ps", name="ps")
            for k in range(KT):
                nc.tensor.matmul(
                    ps[:, 0:CS],
                    lhsT=xT_sb[k][:, t * 128:(t + 1) * 128],
                    rhs=wv_sb[k][:, :],
                    start=(k == 0),
                    stop=(k == KT - 1),
                )
            nc.any.tensor_copy(
                v_sb[t][:, :, 0:D],
                ps[:, 0:CS].rearrange("p (h d) -> p h d", h=HPC),
            )

    # ---------------- stages 2-6: attention + out-proj ----------------
    with (
        tc.tile_pool(name="sT", bufs=1, space="PSUM") as sTp,
        tc.tile_pool(name="outT", bufs=1, space="PSUM") as oTp,
        tc.tile_pool(name="zrep", bufs=1, space="PSUM") as zTp,
        tc.tile_pool(name="yps", bufs=2, space="PSUM") as yp,
        tc.tile_pool(name="pT", bufs=3) as pTp,
        tc.tile_pool(name="small", bufs=4) as smallp,
        tc.tile_pool(name="ystage", bufs=3) as ysp,
    ):
        for h in range(HPC):
            ktile = kT_sb[h // 2]
            qtile = qT_sb[h // 2]
            po = (h % 2) * D
            for qb in range(NQB):
                ngroups = qb + 1
                njt = 4 * ngroups
                oT = oTp.tile([D + 1, QB], F32, tag="oT", name="oT")
                for grp in range(ngroups):
                    sT = sTp.tile([128, 4, QB], F32, tag="sT", name="sT")
                    for m in range(4):
                        jt = grp * 4 + m
                        nc.tensor.matmul(
                            sT[:, m, :],
                            lhsT=ktile[po:po + D, jt * 128:(jt + 1) * 128],
                            rhs=qtile[po:po + D, qb * QB:(qb + 1) * QB],
                            start=True,
                            stop=True,
                        )
                    if grp == ngroups - 1:  # diagonal group: causal mask
                        for m in range(4):
                            nc.vector.tensor_add(sT[:, m, :], sT[:, m, :], mask_sb[:, m, :])
                    pT = pTp.tile([128, 4, QB], F32R, tag="pT", name="pT")
                    nc.scalar.activation(pT[:, :, :], sT[:, :, :], AF.Exp)
                    for m in range(4):
                        jt = grp * 4 + m
                        nc.tensor.matmul(
                            oT[:, :],
                            lhsT=v_sb[jt][:, h, :],
                            rhs=pT[:, m, :],
                            start=(jt == 0),
                            stop=(jt == njt - 1),
                        )
                # normalize: att^T = outT[0:D] * (1/Z), Z = outT[D]
                rz = smallp.tile([1, QB], F32R, tag="rz", name="rz")
                nc.vector.reciprocal(rz[:, :], oT[D:D + 1, :])
                zr = zTp.tile([D, QB], F32, tag="zr", name="zr")
                nc.tensor.matmul(
                    zr[:, :],
                    lhsT=ones_sb[:, :],
                    rhs=rz[:, :],
                    start=True,
                    stop=True,
                )
                zs = smallp.tile([D, QB], F32, tag="zs", name="zs")
                nc.any.tensor_copy(zs[:, :], zr[:, :])
                nc.vector.tensor_mul(
                    attT_sb[h // 2][po:po + D, qb * QB:(qb + 1) * QB],
                    oT[0:D, :],
                    zs[:, :],
                )

        # stage 6: y^T[e, t] = Wp[c, e].T @ att^T[c, t]
        for et in range(C // 128):
            for tc4 in range(NQB):
                yps_t = yp.tile([128, QB], F32, tag="y", name="y")
                for kc in range(2):
                    nc.tensor.matmul(
                        yps_t[:, :],
                        lhsT=wp_sb[kc][:, et * 128:(et + 1) * 128],
                        rhs=attT_sb[kc][:, tc4 * QB:(tc4 + 1) * QB],
                        start=(kc == 0),
                        stop=(kc == 1),
                    )
                ys = ysp.tile([128, QB], F32, tag="ys", name="ys")
                nc.any.tensor_copy(ys[:, :], yps_t[:, :])
                nc.sync.dma_start(
                    out=yT[et * 128:(et + 1) * 128, tc4 * QB:(tc4 + 1) * QB],
                    in_=ys[:, :],
                )


def build_nc():
    from contextlib import ExitStack

    nc = bacc.Bacc("TRN2", target_bir_lowering=False)
    xT = nc.dram_tensor("xT", [C, T], F32R, kind="ExternalInput")
    wqk = nc.dram_tensor("wqk", [C, 2 * CS], F32R, kind="ExternalInput")
    wv = nc.dram_tensor("wv", [C, CS], F32R, kind="ExternalInput")
    bqk = nc.dram_tensor("bqk", [2 * CS, 1], F32, kind="ExternalInput")
    wp = nc.dram_tensor("wp", [CS, C], F32R, kind="ExternalInput")
    masks = nc.dram_tensor("masks", [128, 4, QB], F32, kind="ExternalInput")
    ones4 = nc.dram_tensor("ones4", [128, 4], F32R, kind="ExternalInput")
    ones64 = nc.dram_tensor("ones64", [1, D], F32R, kind="ExternalInput")
    yT = nc.dram_tensor("yT", [C, T], F32, kind="ExternalOutput")
    with tile.TileContext(nc) as tc:
        with nc.allow_low_precision(reason="fp32r matmul inputs; accumulation stays fp32 in PSUM"):
            with ExitStack() as ctx:
                _build_body(nc, tc, ctx, xT, wqk, wv, bqk, wp, masks, ones4, ones64, yT)
    nc.compile()
    return nc


def make_masks():
    r = np.arange(128)[:, None, None]
    m = np.arange(4)[None, :, None]
    c = np.arange(QB)[None, None, :]
    return np.where(128 * m + r <= c, np.float32(0.0), np.float32(NEG)).astype(np.float32)


def make_in_maps(x, W_qkv, b_qkv, W_proj):
    scale = np.float32(1.0 / np.sqrt(D))
    mask_h = make_masks()
    in_maps = []
    for i in range(NCORES):
        b, g = divmod(i, HPC)
        cs0 = g * CS
        wq = W_qkv[:, cs0:cs0 + CS] * scale
        wk = W_qkv[:, C + cs0:C + cs0 + CS]
        bq = b_qkv[cs0:cs0 + CS] * scale
        bk = b_qkv[C + cs0:C + cs0 + CS]
        in_maps.append({
            "xT": np.ascontiguousarray(x[b].T).astype(np.float32),
            "wqk": np.concatenate([wq, wk], axis=1).astype(np.float32),
            "wv": np.ascontiguousarray(W_qkv[:, 2 * C + cs0:2 * C + cs0 + CS]).astype(np.float32),
            "bqk": np.concatenate([bq, bk])[:, None].astype(np.float32),
            "wp": np.ascontiguousarray(W_proj[cs0:cs0 + CS, :]).astype(np.float32),
            "masks": mask_h,
            "ones4": np.ones((128, 4), np.float32),
            "ones64": np.ones((1, D), np.float32),
        })
    return in_maps


_NC_CACHE = None


def _get_nc():
    global _NC_CACHE
    if _NC_CACHE is None:
        _NC_CACHE = build_nc()
    return _NC_CACHE


def gather(results, b_qkv, W_proj, b_proj):
    Y = np.zeros((B, T, C), np.float32)
    for i in range(NCORES):
        Y[i // HPC] += results[i]["yT"].T
    Y += (b_qkv[2 * C:].astype(np.float32) @ W_proj.astype(np.float32)
          + b_proj.astype(np.float32))[None, None, :]
    return Y


def kernel(x, W_qkv, b_qkv, W_proj, b_proj):
    global LAST_RESULT
    x = np.asarray(x, np.float32)
    W_qkv = np.asarray(W_qkv, np.float32)
    b_qkv = np.asarray(b_qkv, np.float32)
    W_proj = np.asarray(W_proj, np.float32)
    b_proj = np.asarray(b_proj, np.float32)

    nc = _get_nc()
    in_maps = make_in_maps(x, W_qkv, b_qkv, W_proj)
    res = run_bass_kernel_spmd(nc, in_maps, list(range(NCORES)), trace=TRACE)
    LAST_RESULT = res
    if TRACE and res.exec_time_ns is not None:
        print(f"HW exec time: {res.exec_time_ns} ns")
    return gather(res.results, b_qkv, W_proj, b_proj)


# revision 9
# speedup vs baseline: 1.3283x; 1.3283x over previous
"""Causal multi-head attention (B=2, T=2048, C=1024, H=16, d=64) on 8 trn2 cores.

Sharding: core i -> (batch b = i//4, head group g = i%4, 4 heads/core).
Data parallel over B, tensor parallel over heads; the out-proj partial sums
(contraction over this core's 256 channels) are reduced on the host during
the gather step, along with b_proj and the analytically-folded V bias.

Device kernel works entirely in [feature, token] (transposed) layout so no
on-device transposes are needed:
  stage 1: Q^T,K^T = (Wqk)^T x^T   (+bias, 1/sqrt(d) folded into Wq host-side)
           V       = x W_v         (natural layout, used as stage-4 lhsT)
  stage 2: S^T[j,q] = K_h^T.T @ Q_h^T   per head, causal tiles only
  stage 3: P^T = exp(S^T + mask)        (no max subtraction; scores are O(20))
  stage 4: outT[65,q] = [V_h | 1]^T.T @ P^T  accumulated over j tiles
           (row 64 = softmax denominator Z)
  stage 5: att^T = outT[0:64] * (1/Z broadcast via PE rank-1)
  stage 6: y^T = Wp.T @ att^T  -> DMA out; host sums partials + transposes.
"""

import numpy as np

import concourse.bass as bass
import concourse.mybir as mybir
from concourse import bacc
import concourse.tile as tile
from concourse.bass_utils import run_bass_kernel_spmd

B, T, C, H, D = 2, 2048, 1024, 16, 64
NCORES = 8
HPC = 4            # heads per core
CS = HPC * D       # 256 channels per core (per Q/K/V block)
KT = C // 128      # 8 contraction tiles for the projections
NT = T // 128      # 16 token tiles of 128
QB = 512           # query block (psum bank width in fp32)
NQB = T // QB      # 4 query blocks
NEG = -1e9

F32 = mybir.dt.float32
F32R = mybir.dt.float32r  # PE full rate at N>=256; matmul-input dtype

TRACE = False
LAST_RESULT = None


def _build_body(nc, tc, ctx, xT, wqk, wv, bqk, wp, masks, ones4, yT):
    AF = mybir.ActivationFunctionType

    persist = ctx.enter_context(tc.tile_pool(name="persist", bufs=1))

    wqk_sb = [persist.tile([128, 2 * CS], F32R, tag=f"wqk{k}", name=f"wqk{k}") for k in range(KT)]
    wv_sb = [persist.tile([128, CS], F32R, tag=f"wv{k}", name=f"wv{k}") for k in range(KT)]
    bqk_sb = [persist.tile([128, 1], F32, tag=f"bqk{c}", name=f"bqk{c}") for c in range(4)]
    wp_sb = [persist.tile([128, C], F32R, tag=f"wp{k}", name=f"wp{k}") for k in range(2)]
    mask_sb = persist.tile([128, 4, QB], F32, tag="mask", name="mask_sb")
    qT_sb = [persist.tile([128, T], F32R, tag=f"qT{i}", name=f"qT{i}") for i in range(2)]
    kT_sb = [persist.tile([128, T], F32R, tag=f"kT{i}", name=f"kT{i}") for i in range(2)]
    v_sb = [persist.tile([128, HPC, D + 1], F32R, tag=f"v{t}", name=f"v{t}") for t in range(NT)]
    attT_sb = [persist.tile([128, T], F32R, tag=f"attT{i}", name=f"attT{i}") for i in range(2)]

    for k in range(KT):
        nc.sync.dma_start(out=wqk_sb[k][:, :], in_=wqk[k * 128:(k + 1) * 128, :])
        nc.sync.dma_start(out=wv_sb[k][:, :], in_=wv[k * 128:(k + 1) * 128, :])
    for c4 in range(4):
        nc.sync.dma_start(out=bqk_sb[c4][:, :], in_=bqk[c4 * 128:(c4 + 1) * 128, :])
    for k in range(2):
        nc.sync.dma_start(out=wp_sb[k][:, :], in_=wp[k * 128:(k + 1) * 128, :])
    nc.sync.dma_start(out=mask_sb[:, :, :], in_=masks[:, :, :])
    for t in range(NT):
        nc.sync.dma_start(out=v_sb[t][:, :, D], in_=ones4[:, :])

    # ---------------- stage 1: projections ----------------
    with (
        tc.tile_pool(name="xpool", bufs=1) as xpool,
        tc.tile_pool(name="s1psum", bufs=3, space="PSUM") as s1p,
    ):
        xT_sb = [xpool.tile([128, T], F32R, tag=f"xT{k}", name=f"xT{k}") for k in range(KT)]
        for k in range(KT):
            nc.sync.dma_start(out=xT_sb[k][:, :], in_=xT[k * 128:(k + 1) * 128, :])

        # Q^T (c-tiles 0,1) and K^T (c-tiles 2,3): out[c, t] over k
        for ct in range(4):
            dst = qT_sb[ct] if ct < 2 else kT_sb[ct - 2]
            for tc4 in range(NQB):
                ps = s1p.tile([128, QB], F32, tag="ps", name="ps")
                for k in range(KT):
                    nc.tensor.matmul(
                        ps[:, :],
                        lhsT=wqk_sb[k][:, ct * 128:(ct + 1) * 128],
                        rhs=xT_sb[k][:, tc4 * QB:(tc4 + 1) * QB],
                        start=(k == 0),
                        stop=(k == KT - 1),
                    )
                nc.vector.tensor_scalar_add(
                    dst[:, tc4 * QB:(tc4 + 1) * QB], ps[:, :], bqk_sb[ct][:, :]
                )

        # V natural layout: V[t, c] over k; ones column appended per head
        for t in range(NT):
            ps = s1p.tile([128, QB], F32, tag="ps", name="ps")
            for k in range(KT):
                nc.tensor.matmul(
                    ps[:, 0:CS],
                    lhsT=xT_sb[k][:, t * 128:(t + 1) * 128],
                    rhs=wv_sb[k][:, :],
                    start=(k == 0),
                    stop=(k == KT - 1),
                )
            nc.any.tensor_copy(
                v_sb[t][:, :, 0:D],
                ps[:, 0:CS].rearrange("p (h d) -> p h d", h=HPC),
            )

    # ---------------- stages 2-6: attention + out-proj ----------------
    with (
        tc.tile_pool(name="sT", bufs=2, space="PSUM") as sTp,
        tc.tile_pool(name="outT", bufs=2, space="PSUM") as oTp,
        tc.tile_pool(name="yps", bufs=2, space="PSUM") as yp,
        tc.tile_pool(name="pT", bufs=6) as pTp,
        tc.tile_pool(name="small", bufs=4) as smallp,
        tc.tile_pool(name="ystage", bufs=3) as ysp,
    ):
        for h in range(HPC):
            ktile = kT_sb[h // 2]
            qtile = qT_sb[h // 2]
            po = (h % 2) * D
            for qb in range(NQB):
                ngr = 2 * (qb + 1)   # groups of 2 j-tiles each
                njt = 4 * (qb + 1)
                oT = oTp.tile([D + 1, QB], F32, tag="oT", name="oT")
                for grp in range(ngr):
                    sT = sTp.tile([128, 2, QB], F32, tag="sT", name="sT")
                    for m in range(2):
                        jt = grp * 2 + m
                        nc.tensor.matmul(
                            sT[:, m, :],
                            lhsT=ktile[po:po + D, jt * 128:(jt + 1) * 128],
                            rhs=qtile[po:po + D, qb * QB:(qb + 1) * QB],
                            start=True,
                            stop=True,
                        )
                    if grp >= ngr - 2:  # diagonal groups: causal mask
                        for m in range(2):
                            jt = grp * 2 + m
                            dm = jt - 4 * qb
                            nc.vector.tensor_add(sT[:, m, :], sT[:, m, :], mask_sb[:, dm, :])
                    pT = pTp.tile([128, 2, QB], F32R, tag="pT", name="pT")
                    nc.scalar.activation(pT[:, :, :], sT[:, :, :], AF.Exp)
                    for m in range(2):
                        jt = grp * 2 + m
                        nc.tensor.matmul(
                            oT[:, :],
                            lhsT=v_sb[jt][:, h, :],
                            rhs=pT[:, m, :],
                            start=(jt == 0),
                            stop=(jt == njt - 1),
                        )
                # normalize: att^T = outT[0:D] * (1/Z), Z = outT[D]
                rz = smallp.tile([1, QB], F32, tag="rz", name="rz")
                nc.vector.reciprocal(rz[:, :], oT[D:D + 1, :])
                zs = smallp.tile([D, QB], F32, tag="zs", name="zs")
                nc.gpsimd.partition_broadcast(zs[:, :], rz[:, :], channels=D)
                nc.vector.tensor_mul(
                    attT_sb[h // 2][po:po + D, qb * QB:(qb + 1) * QB],
                    oT[0:D, :],
                    zs[:, :],
                )

        # stage 6: y^T[e, t] = Wp[c, e].T @ att^T[c, t]
        for et in range(C // 128):
            for tc4 in range(NQB):
                yps_t = yp.tile([128, QB], F32, tag="y", name="y")
                for kc in range(2):
                    nc.tensor.matmul(
                        yps_t[:, :],
                        lhsT=wp_sb[kc][:, et * 128:(et + 1) * 128],
                        rhs=attT_sb[kc][:, tc4 * QB:(tc4 + 1) * QB],
                        start=(kc == 0),
                        stop=(kc == 1),
                    )
                ys = ysp.tile([128, QB], F32, tag="ys", name="ys")
                nc.any.tensor_copy(ys[:, :], yps_t[:, :])
                nc.sync.dma_start(
                    out=yT[et * 128:(et + 1) * 128, tc4 * QB:(tc4 + 1) * QB],
                    in_=ys[:, :],
                )


def build_nc():
    from contextlib import ExitStack

    nc = bacc.Bacc("TRN2", target_bir_lowering=False)
    xT = nc.dram_tensor("xT", [C, T], F32R, kind="ExternalInput")
    wqk = nc.dram_tensor("wqk", [C, 2 * CS], F32R, kind="ExternalInput")
    wv = nc.dram_tensor("wv", [C, CS], F32R, kind="ExternalInput")
    bqk = nc.dram_tensor("bqk", [2 * CS, 1], F32, kind="ExternalInput")
    wp = nc.dram_tensor("wp", [CS, C], F32R, kind="ExternalInput")
    masks = nc.dram_tensor("masks", [128, 4, QB], F32, kind="ExternalInput")
    ones4 = nc.dram_tensor("ones4", [128, 4], F32R, kind="ExternalInput")
    yT = nc.dram_tensor("yT", [C, T], F32, kind="ExternalOutput")
    with tile.TileContext(nc) as tc:
        with nc.allow_low_precision(reason="fp32r matmul inputs; accumulation stays fp32 in PSUM"):
            with ExitStack() as ctx:
                _build_body(nc, tc, ctx, xT, wqk, wv, bqk, wp, masks, ones4, yT)
    nc.compile()
    return nc


def make_masks():
    r = np.arange(128)[:, None, None]
    m = np.arange(4)[None, :, None]
    c = np.arange(QB)[None, None, :]
    return np.where(128 * m + r <= c, np.float32(0.0), np.float32(NEG)).astype(np.float32)


def make_in_maps(x, W_qkv, b_qkv, W_proj):
    scale = np.float32(1.0 / np.sqrt(D))
    mask_h = make_masks()
    in_maps = []
    for i in range(NCORES):
        b, g = divmod(i, HPC)
        cs0 = g * CS
        wq = W_qkv[:, cs0:cs0 + CS] * scale
        wk = W_qkv[:, C + cs0:C + cs0 + CS]
        bq = b_qkv[cs0:cs0 + CS] * scale
        bk = b_qkv[C + cs0:C + cs0 + CS]
        in_maps.append({
            "xT": np.ascontiguousarray(x[b].T).astype(np.float32),
            "wqk": np.concatenate([wq, wk], axis=1).astype(np.float32),
            "wv": np.ascontiguousarray(W_qkv[:, 2 * C + cs0:2 * C + cs0 + CS]).astype(np.float32),
            "bqk": np.concatenate([bq, bk])[:, None].astype(np.float32),
            "wp": np.ascontiguousarray(W_proj[cs0:cs0 + CS, :]).astype(np.float32),
            "masks": mask_h,
            "ones4": np.ones((128, 4), np.float32),
        })
    return in_maps


_NC_CACHE = None


def _get_nc():
    global _NC_CACHE
    if _NC_CACHE is None:
        _NC_CACHE = build_nc()
    return _NC_CACHE


def gather(results, b_qkv, W_proj, b_proj):
    Y = np.zeros((B, T, C), np.float32)
    for i in range(NCORES):
        Y[i // HPC] += results[i]["yT"].T
    Y += (b_qkv[2 * C:].astype(np.float32) @ W_proj.astype(np.float32)
          + b_proj.astype(np.float32))[None, None, :]
    return Y


def kernel(x, W_qkv, b_qkv, W_proj, b_proj):
    global LAST_RESULT
    x = np.asarray(x, np.float32)
    W_qkv = np.asarray(W_qkv, np.float32)
    b_qkv = np.asarray(b_qkv, np.float32)
    W_proj = np.asarray(W_proj, np.float32)
    b_proj = np.asarray(b_proj, np.float32)

    nc = _get_nc()
    in_maps = make_in_maps(x, W_qkv, b_qkv, W_proj)
    res = run_bass_kernel_spmd(nc, in_maps, list(range(NCORES)), trace=TRACE)
    LAST_RESULT = res
    if TRACE and res.exec_time_ns is not None:
        print(f"HW exec time: {res.exec_time_ns} ns")
    return gather(res.results, b_qkv, W_proj, b_proj)


# revision 10
# speedup vs baseline: 1.4014x; 1.0550x over previous
"""Causal multi-head attention (B=2, T=2048, C=1024, H=16, d=64) on 8 trn2 cores.

Sharding: core i -> (batch b = i//4, head group g = i%4, 4 heads/core).
Data parallel over B, tensor parallel over heads; the out-proj partial sums
(contraction over this core's 256 channels) are reduced on the host during
the gather step, along with b_proj and the analytically-folded V bias.

Device kernel works entirely in [feature, token] (transposed) layout so no
on-device transposes are needed:
  stage 1: Q^T,K^T = (Wqk)^T x^T   (+bias, 1/sqrt(d) folded into Wq host-side)
           V       = x W_v         (natural layout, used as stage-4 lhsT)
  stage 2: S^T[j,q] = K_h^T.T @ Q_h^T   per head, causal tiles only
  stage 3: P^T = exp(S^T + mask)        (no max subtraction; scores are O(20))
  stage 4: outT[65,q] = [V_h | 1]^T.T @ P^T  accumulated over j tiles
           (row 64 = softmax denominator Z)
  stage 5: att^T = outT[0:64] * (1/Z broadcast via PE rank-1)
  stage 6: y^T = Wp.T @ att^T  -> DMA out; host sums partials + transposes.
"""

import numpy as np

import concourse.bass as bass
import concourse.mybir as mybir
from concourse import bacc
import concourse.tile as tile
from concourse.bass_utils import run_bass_kernel_spmd

B, T, C, H, D = 2, 2048, 1024, 16, 64
NCORES = 8
HPC = 4            # heads per core
CS = HPC * D       # 256 channels per core (per Q/K/V block)
KT = C // 128      # 8 contraction tiles for the projections
NT = T // 128      # 16 token tiles of 128
QB = 512           # query block (psum bank width in fp32)
NQB = T // QB      # 4 query blocks
NEG = -1e9

F32 = mybir.dt.float32
F32R = mybir.dt.float32r  # PE full rate at N>=256; matmul-input dtype

TRACE = False
LAST_RESULT = None


def _build_body(nc, tc, ctx, xT, wqk, wv, bqk, wp, masks, ones4, yT):
    AF = mybir.ActivationFunctionType

    persist = ctx.enter_context(tc.tile_pool(name="persist", bufs=1))

    wqk_sb = [persist.tile([128, 2 * CS], F32R, tag=f"wqk{k}", name=f"wqk{k}") for k in range(KT)]
    wv_sb = [persist.tile([128, CS], F32R, tag=f"wv{k}", name=f"wv{k}") for k in range(KT)]
    bqk_sb = [persist.tile([128, 1], F32, tag=f"bqk{c}", name=f"bqk{c}") for c in range(4)]
    wp_sb = [persist.tile([128, C], F32R, tag=f"wp{k}", name=f"wp{k}") for k in range(2)]
    mask_sb = persist.tile([128, 4, QB], F32, tag="mask", name="mask_sb")
    qT_sb = [persist.tile([128, T], F32R, tag=f"qT{i}", name=f"qT{i}") for i in range(2)]
    kT_sb = [persist.tile([128, T], F32R, tag=f"kT{i}", name=f"kT{i}") for i in range(2)]
    v_sb = [persist.tile([128, HPC, D + 1], F32R, tag=f"v{t}", name=f"v{t}") for t in range(NT)]
    attT_sb = [persist.tile([128, T], F32R, tag=f"attT{i}", name=f"attT{i}") for i in range(2)]

    for k in range(KT):
        nc.sync.dma_start(out=wqk_sb[k][:, :], in_=wqk[k * 128:(k + 1) * 128, :])
        nc.sync.dma_start(out=wv_sb[k][:, :], in_=wv[k * 128:(k + 1) * 128, :])
    for c4 in range(4):
        nc.sync.dma_start(out=bqk_sb[c4][:, :], in_=bqk[c4 * 128:(c4 + 1) * 128, :])
    for k in range(2):
        nc.sync.dma_start(out=wp_sb[k][:, :], in_=wp[k * 128:(k + 1) * 128, :])
    nc.sync.dma_start(out=mask_sb[:, :, :], in_=masks[:, :, :])
    for t in range(NT):
        nc.sync.dma_start(out=v_sb[t][:, :, D], in_=ones4[:, :])

    # ---------------- stage 1: projections ----------------
    with (
        tc.tile_pool(name="xpool", bufs=1) as xpool,
        tc.tile_pool(name="s1psum", bufs=3, space="PSUM") as s1p,
    ):
        xT_sb = [xpool.tile([128, T], F32R, tag=f"xT{k}", name=f"xT{k}") for k in range(KT)]
        for k in range(KT):
            nc.sync.dma_start(out=xT_sb[k][:, :], in_=xT[k * 128:(k + 1) * 128, :])

        # Q^T (c-tiles 0,1) and K^T (c-tiles 2,3): out[c, t] over k
        for ct in range(4):
            dst = qT_sb[ct] if ct < 2 else kT_sb[ct - 2]
            for tc4 in range(NQB):
                ps = s1p.tile([128, QB], F32, tag="ps", name="ps")
                for k in range(KT):
                    nc.tensor.matmul(
                        ps[:, :],
                        lhsT=wqk_sb[k][:, ct * 128:(ct + 1) * 128],
                        rhs=xT_sb[k][:, tc4 * QB:(tc4 + 1) * QB],
                        start=(k == 0),
                        stop=(k == KT - 1),
                    )
                nc.vector.tensor_scalar_add(
                    dst[:, tc4 * QB:(tc4 + 1) * QB], ps[:, :], bqk_sb[ct][:, :]
                )

        # V natural layout: V[t, c] over k; ones column appended per head
        for t in range(NT):
            ps = s1p.tile([128, QB], F32, tag="ps", name="ps")
            for k in range(KT):
                nc.tensor.matmul(
                    ps[:, 0:CS],
                    lhsT=xT_sb[k][:, t * 128:(t + 1) * 128],
                    rhs=wv_sb[k][:, :],
                    start=(k == 0),
                    stop=(k == KT - 1),
                )
            nc.any.tensor_copy(
                v_sb[t][:, :, 0:D],
                ps[:, 0:CS].rearrange("p (h d) -> p h d", h=HPC),
            )

    # ---------------- stages 2-6: attention + out-proj ----------------
    with (
        tc.tile_pool(name="sT", bufs=3, space="PSUM") as sTp,
        tc.tile_pool(name="outT", bufs=1, space="PSUM") as oTp,
        tc.tile_pool(name="yps", bufs=1, space="PSUM") as yp,
        tc.tile_pool(name="pT", bufs=6) as pTp,
        tc.tile_pool(name="small", bufs=4) as smallp,
        tc.tile_pool(name="ystage", bufs=4) as ysp,
    ):
        for qb in range(NQB):
            for h in range(HPC):
                ktile = kT_sb[h // 2]
                qtile = qT_sb[h // 2]
                po = (h % 2) * D
                ngr = 2 * (qb + 1)   # groups of 2 j-tiles each
                njt = 4 * (qb + 1)
                oT = oTp.tile([D + 1, QB], F32, tag="oT", name="oT")
                for grp in range(ngr):
                    sT = sTp.tile([128, 2, QB], F32, tag="sT", name="sT")
                    for m in range(2):
                        jt = grp * 2 + m
                        nc.tensor.matmul(
                            sT[:, m, :],
                            lhsT=ktile[po:po + D, jt * 128:(jt + 1) * 128],
                            rhs=qtile[po:po + D, qb * QB:(qb + 1) * QB],
                            start=True,
                            stop=True,
                        )
                    if grp >= ngr - 2:  # diagonal groups: causal mask
                        for m in range(2):
                            jt = grp * 2 + m
                            dm = jt - 4 * qb
                            nc.vector.tensor_add(sT[:, m, :], sT[:, m, :], mask_sb[:, dm, :])
                    pT = pTp.tile([128, 2, QB], F32R, tag="pT", name="pT")
                    nc.scalar.activation(pT[:, :, :], sT[:, :, :], AF.Exp)
                    for m in range(2):
                        jt = grp * 2 + m
                        nc.tensor.matmul(
                            oT[:, :],
                            lhsT=v_sb[jt][:, h, :],
                            rhs=pT[:, m, :],
                            start=(jt == 0),
                            stop=(jt == njt - 1),
                        )
                # normalize: att^T = outT[0:D] * (1/Z), Z = outT[D]
                rz = smallp.tile([1, QB], F32, tag="rz", name="rz")
                nc.vector.reciprocal(rz[:, :], oT[D:D + 1, :])
                zs = smallp.tile([D, QB], F32, tag="zs", name="zs")
                nc.gpsimd.partition_broadcast(zs[:, :], rz[:, :], channels=D)
                nc.vector.tensor_mul(
                    attT_sb[h // 2][po:po + D, qb * QB:(qb + 1) * QB],
                    oT[0:D, :],
                    zs[:, :],
                )

            # stage 6 for this q-block: y^T[e, qb] = Wp.T @ att^T[:, qb]
            for et in range(C // 128):
                yps_t = yp.tile([128, QB], F32, tag="y", name="y")
                for kc in range(2):
                    nc.tensor.matmul(
                        yps_t[:, :],
                        lhsT=wp_sb[kc][:, et * 128:(et + 1) * 128],
                        rhs=attT_sb[kc][:, qb * QB:(qb + 1) * QB],
                        start=(kc == 0),
                        stop=(kc == 1),
                    )
                ys = ysp.tile([128, QB], F32, tag="ys", name="ys")
                nc.any.tensor_copy(ys[:, :], yps_t[:, :])
                nc.sync.dma_start(
                    out=yT[et * 128:(et + 1) * 128, qb * QB:(qb + 1) * QB],
                    in_=ys[:, :],
                )


def build_nc():
    from contextlib import ExitStack

    nc = bacc.Bacc("TRN2", target_bir_lowering=False)
    xT = nc.dram_tensor("xT", [C, T], F32R, kind="ExternalInput")
    wqk = nc.dram_tensor("wqk", [C, 2 * CS], F32R, kind="ExternalInput")
    wv = nc.dram_tensor("wv", [C, CS], F32R, kind="ExternalInput")
    bqk = nc.dram_tensor("bqk", [2 * CS, 1], F32, kind="ExternalInput")
    wp = nc.dram_tensor("wp", [CS, C], F32R, kind="ExternalInput")
    masks = nc.dram_tensor("masks", [128, 4, QB], F32, kind="ExternalInput")
    ones4 = nc.dram_tensor("ones4", [128, 4], F32R, kind="ExternalInput")
    yT = nc.dram_tensor("yT", [C, T], F32, kind="ExternalOutput")
    with tile.TileContext(nc) as tc:
        with nc.allow_low_precision(reason="fp32r matmul inputs; accumulation stays fp32 in PSUM"):
            with ExitStack() as ctx:
                _build_body(nc, tc, ctx, xT, wqk, wv, bqk, wp, masks, ones4, yT)
    nc.compile()
    return nc


def make_masks():
    r = np.arange(128)[:, None, None]
    m = np.arange(4)[None, :, None]
    c = np.arange(QB)[None, None, :]
    return np.where(128 * m + r <= c, np.float32(0.0), np.float32(NEG)).astype(np.float32)


def make_in_maps(x, W_qkv, b_qkv, W_proj):
    scale = np.float32(1.0 / np.sqrt(D))
    mask_h = make_masks()
    in_maps = []
    for i in range(NCORES):
        b, g = divmod(i, HPC)
        cs0 = g * CS
        wq = W_qkv[:, cs0:cs0 + CS] * scale
        wk = W_qkv[:, C + cs0:C + cs0 + CS]
        bq = b_qkv[cs0:cs0 + CS] * scale
        bk = b_qkv[C + cs0:C + cs0 + CS]
        in_maps.append({
            "xT": np.ascontiguousarray(x[b].T).astype(np.float32),
            "wqk": np.concatenate([wq, wk], axis=1).astype(np.float32),
            "wv": np.ascontiguousarray(W_qkv[:, 2 * C + cs0:2 * C + cs0 + CS]).astype(np.float32),
            "bqk": np.concatenate([bq, bk])[:, None].astype(np.float32),
            "wp": np.ascontiguousarray(W_proj[cs0:cs0 + CS, :]).astype(np.float32),
            "masks": mask_h,
            "ones4": np.ones((128, 4), np.float32),
        })
    return in_maps


_NC_CACHE = None


def _get_nc():
    global _NC_CACHE
    if _NC_CACHE is None:
        _NC_CACHE = build_nc()
    return _NC_CACHE


def gather(results, b_qkv, W_proj, b_proj):
    Y = np.zeros((B, T, C), np.float32)
    for i in range(NCORES):
        Y[i // HPC] += results[i]["yT"].T
    Y += (b_qkv[2 * C:].astype(np.float32) @ W_proj.astype(np.float32)
          + b_proj.astype(np.float32))[None, None, :]
    return Y


def kernel(x, W_qkv, b_qkv, W_proj, b_proj):
    global LAST_RESULT
    x = np.asarray(x, np.float32)
    W_qkv = np.asarray(W_qkv, np.float32)
    b_qkv = np.asarray(b_qkv, np.float32)
    W_proj = np.asarray(W_proj, np.float32)
    b_proj = np.asarray(b_proj, np.float32)

    nc = _get_nc()
    in_maps = make_in_maps(x, W_qkv, b_qkv, W_proj)
    res = run_bass_kernel_spmd(nc, in_maps, list(range(NCORES)), trace=TRACE)
    LAST_RESULT = res
    if TRACE and res.exec_time_ns is not None:
        print(f"HW exec time: {res.exec_time_ns} ns")
    return gather(res.results, b_qkv, W_proj, b_proj)


# revision 12
# speedup vs baseline: 1.6098x; 1.1487x over previous
"""Causal multi-head attention (B=2, T=2048, C=1024, H=16, d=64) on 8 trn2 cores.

Sharding: core i -> (batch b = i//4, head group g = i%4, 4 heads/core).
Data parallel over B, tensor parallel over heads; the out-proj partial sums
(contraction over this core's 256 channels) are reduced on the host during
the gather step, along with b_proj and the analytically-folded V bias.

Device kernel works entirely in [feature, token] (transposed) layout so no
on-device transposes are needed:
  stage 1: Q^T,K^T = (Wqk)^T x^T   (+bias, 1/sqrt(d) folded into Wq host-side)
           V       = x W_v         (natural layout, used as stage-4 lhsT)
  stage 2: S^T[j,q] = K_h^T.T @ Q_h^T   per head, causal tiles only
  stage 3: P^T = exp(S^T + mask)        (no max subtraction; scores are O(20))
  stage 4: outT[65,q] = [V_h | 1]^T.T @ P^T  accumulated over j tiles
           (row 64 = softmax denominator Z)
  stage 5: att^T = outT[0:64] * (1/Z broadcast via PE rank-1)
  stage 6: y^T = Wp.T @ att^T  -> DMA out; host sums partials + transposes.
"""

import numpy as np

import concourse.bass as bass
import concourse.mybir as mybir
from concourse import bacc
import concourse.tile as tile
from concourse.bass_utils import run_bass_kernel_spmd

B, T, C, H, D = 2, 2048, 1024, 16, 64
NCORES = 8
HPC = 4            # heads per core
CS = HPC * D       # 256 channels per core (per Q/K/V block)
KT = C // 128      # 8 contraction tiles for the projections
NT = T // 128      # 16 token tiles of 128
QB = 512           # query block (psum bank width in fp32)
NQB = T // QB      # 4 query blocks
NEG = -1e9

F32 = mybir.dt.float32
F32R = mybir.dt.float32r  # PE full rate at N>=256; matmul-input dtype

TRACE = False
LAST_RESULT = None


def _build_body(nc, tc, ctx, xT, wqk, wv, bqk, wp, masks, ones4, yT):
    AF = mybir.ActivationFunctionType

    persist = ctx.enter_context(tc.tile_pool(name="persist", bufs=1))

    wqk_sb = [persist.tile([128, 2 * CS], F32R, tag=f"wqk{k}", name=f"wqk{k}") for k in range(KT)]
    wv_sb = [persist.tile([128, CS], F32R, tag=f"wv{k}", name=f"wv{k}") for k in range(KT)]
    bqk_sb = [persist.tile([128, 1], F32, tag=f"bqk{c}", name=f"bqk{c}") for c in range(4)]
    wp_sb = [persist.tile([128, C], F32R, tag=f"wp{k}", name=f"wp{k}") for k in range(2)]
    mask_sb = persist.tile([128, 4, QB], F32, tag="mask", name="mask_sb")
    qT_sb = [persist.tile([128, T], F32R, tag=f"qT{i}", name=f"qT{i}") for i in range(2)]
    kT_sb = [persist.tile([128, T], F32R, tag=f"kT{i}", name=f"kT{i}") for i in range(2)]
    v_sb = [persist.tile([128, HPC, D + 1], F32R, tag=f"v{t}", name=f"v{t}") for t in range(NT)]
    attT_sb = [persist.tile([128, T], F32R, tag=f"attT{i}", name=f"attT{i}") for i in range(2)]

    for k in range(KT):
        nc.sync.dma_start(out=wqk_sb[k][:, :], in_=wqk[k * 128:(k + 1) * 128, :])
        nc.sync.dma_start(out=wv_sb[k][:, :], in_=wv[k * 128:(k + 1) * 128, :])
    for c4 in range(4):
        nc.sync.dma_start(out=bqk_sb[c4][:, :], in_=bqk[c4 * 128:(c4 + 1) * 128, :])
    for k in range(2):
        nc.sync.dma_start(out=wp_sb[k][:, :], in_=wp[k * 128:(k + 1) * 128, :])
    nc.sync.dma_start(out=mask_sb[:, :, :], in_=masks[:, :, :])
    for t in range(NT):
        nc.sync.dma_start(out=v_sb[t][:, :, D], in_=ones4[:, :])

    # ---------------- stage 1: projections ----------------
    with (
        tc.tile_pool(name="xpool", bufs=1) as xpool,
        tc.tile_pool(name="s1psum", bufs=3, space="PSUM") as s1p,
    ):
        xT_sb = [xpool.tile([128, T], F32R, tag=f"xT{k}", name=f"xT{k}") for k in range(KT)]
        for k in range(KT):
            nc.sync.dma_start(out=xT_sb[k][:, :], in_=xT[k * 128:(k + 1) * 128, :])

        # Q^T (c-tiles 0,1) and K^T (c-tiles 2,3): out[c, t] over k
        for ct in range(4):
            dst = qT_sb[ct] if ct < 2 else kT_sb[ct - 2]
            for tc4 in range(NQB):
                ps = s1p.tile([128, QB], F32, tag="ps", name="ps")
                for k in range(KT):
                    nc.tensor.matmul(
                        ps[:, :],
                        lhsT=wqk_sb[k][:, ct * 128:(ct + 1) * 128],
                        rhs=xT_sb[k][:, tc4 * QB:(tc4 + 1) * QB],
                        start=(k == 0),
                        stop=(k == KT - 1),
                    )
                nc.vector.tensor_scalar_add(
                    dst[:, tc4 * QB:(tc4 + 1) * QB], ps[:, :], bqk_sb[ct][:, :]
                )

        # V natural layout: V[t, c] over k; ones column appended per head
        for t in range(NT):
            ps = s1p.tile([128, QB], F32, tag="ps", name="ps")
            for k in range(KT):
                nc.tensor.matmul(
                    ps[:, 0:CS],
                    lhsT=xT_sb[k][:, t * 128:(t + 1) * 128],
                    rhs=wv_sb[k][:, :],
                    start=(k == 0),
                    stop=(k == KT - 1),
                )
            nc.any.tensor_copy(
                v_sb[t][:, :, 0:D],
                ps[:, 0:CS].rearrange("p (h d) -> p h d", h=HPC),
            )

    # ---------------- stages 2-6: attention + out-proj ----------------
    with (
        tc.tile_pool(name="sT", bufs=3, space="PSUM") as sTp,
        tc.tile_pool(name="outT", bufs=1, space="PSUM") as oTp,
        tc.tile_pool(name="yps", bufs=1, space="PSUM") as yp,
        tc.tile_pool(name="pT", bufs=6) as pTp,
        tc.tile_pool(name="small", bufs=4) as smallp,
        tc.tile_pool(name="ystage", bufs=4) as ysp,
    ):
        for qb in range(NQB):
            for h in range(HPC):
                ktile = kT_sb[h // 2]
                qtile = qT_sb[h // 2]
                po = (h % 2) * D
                ngr = 2 * (qb + 1)   # groups of 2 j-tiles each
                njt = 4 * (qb + 1)
                oT = oTp.tile([D + 1, QB], F32, tag="oT", name="oT")
                for grp in range(ngr):
                    sT = sTp.tile([128, 2, QB], F32, tag="sT", name="sT")
                    for m in range(2):
                        jt = grp * 2 + m
                        nc.tensor.matmul(
                            sT[:, m, :],
                            lhsT=ktile[po:po + D, jt * 128:(jt + 1) * 128],
                            rhs=qtile[po:po + D, qb * QB:(qb + 1) * QB],
                            start=True,
                            stop=True,
                        )
                    if grp >= ngr - 2:  # diagonal groups: causal mask
                        for m in range(2):
                            jt = grp * 2 + m
                            dm = jt - 4 * qb
                            nc.vector.tensor_add(sT[:, m, :], sT[:, m, :], mask_sb[:, dm, :])
                    pT = pTp.tile([128, 2, QB], F32R, tag="pT", name="pT")
                    nc.scalar.activation(pT[:, :, :], sT[:, :, :], AF.Exp)
                    for m in range(2):
                        jt = grp * 2 + m
                        nc.tensor.matmul(
                            oT[:, :],
                            lhsT=v_sb[jt][:, h, :],
                            rhs=pT[:, m, :],
                            start=(jt == 0),
                            stop=(jt == njt - 1),
                        )
                # normalize: att^T = outT[0:D] * (1/Z), Z = outT[D]
                zrow = smallp.tile([1, QB], F32, tag="zrow", name="zrow")
                nc.scalar.copy(zrow[:, :], oT[D:D + 1, :])
                rz = smallp.tile([1, QB], F32, tag="rz", name="rz")
                nc.vector.reciprocal_approx_fast(out=rz[:, :], in_=zrow[:, :])
                zs = smallp.tile([D, QB], F32, tag="zs", name="zs")
                nc.gpsimd.partition_broadcast(zs[:, :], rz[:, :], channels=D)
                nc.vector.tensor_mul(
                    attT_sb[h // 2][po:po + D, qb * QB:(qb + 1) * QB],
                    oT[0:D, :],
                    zs[:, :],
                )

            # stage 6 for this q-block: y^T[e, qb] = Wp.T @ att^T[:, qb]
            for et in range(C // 128):
                yps_t = yp.tile([128, QB], F32, tag="y", name="y")
                for kc in range(2):
                    nc.tensor.matmul(
                        yps_t[:, :],
                        lhsT=wp_sb[kc][:, et * 128:(et + 1) * 128],
                        rhs=attT_sb[kc][:, qb * QB:(qb + 1) * QB],
                        start=(kc == 0),
                        stop=(kc == 1),
                    )
                ys = ysp.tile([128, QB], F32, tag="ys", name="ys")
                nc.any.tensor_copy(ys[:, :], yps_t[:, :])
                nc.sync.dma_start(
                    out=yT[et * 128:(et + 1) * 128, qb * QB:(qb + 1) * QB],
                    in_=ys[:, :],
                )


def build_nc():
    from contextlib import ExitStack

    nc = bacc.Bacc("TRN2", target_bir_lowering=False)
    xT = nc.dram_tensor("xT", [C, T], F32R, kind="ExternalInput")
    wqk = nc.dram_tensor("wqk", [C, 2 * CS], F32R, kind="ExternalInput")
    wv = nc.dram_tensor("wv", [C, CS], F32R, kind="ExternalInput")
    bqk = nc.dram_tensor("bqk", [2 * CS, 1], F32, kind="ExternalInput")
    wp = nc.dram_tensor("wp", [CS, C], F32R, kind="ExternalInput")
    masks = nc.dram_tensor("masks", [128, 4, QB], F32, kind="ExternalInput")
    ones4 = nc.dram_tensor("ones4", [128, 4], F32R, kind="ExternalInput")
    yT = nc.dram_tensor("yT", [C, T], F32, kind="ExternalOutput")
    with tile.TileContext(nc) as tc:
        with nc.allow_low_precision(reason="fp32r matmul inputs; accumulation stays fp32 in PSUM"):
            with ExitStack() as ctx:
                _build_body(nc, tc, ctx, xT, wqk, wv, bqk, wp, masks, ones4, yT)
    nc.compile()
    return nc


def make_masks():
    r = np.arange(128)[:, None, None]
    m = np.arange(4)[None, :, None]
    c = np.arange(QB)[None, None, :]
    return np.where(128 * m + r <= c, np.float32(0.0), np.float32(NEG)).astype(np.float32)


def make_in_maps(x, W_qkv, b_qkv, W_proj):
    scale = np.float32(1.0 / np.sqrt(D))
    mask_h = make_masks()
    in_maps = []
    for i in range(NCORES):
        b, g = divmod(i, HPC)
        cs0 = g * CS
        wq = W_qkv[:, cs0:cs0 + CS] * scale
        wk = W_qkv[:, C + cs0:C + cs0 + CS]
        bq = b_qkv[cs0:cs0 + CS] * scale
        bk = b_qkv[C + cs0:C + cs0 + CS]
        in_maps.append({
            "xT": np.ascontiguousarray(x[b].T).astype(np.float32),
            "wqk": np.concatenate([wq, wk], axis=1).astype(np.float32),
            "wv": np.ascontiguousarray(W_qkv[:, 2 * C + cs0:2 * C + cs0 + CS]).astype(np.float32),
            "bqk": np.concatenate([bq, bk])[:, None].astype(np.float32),
            "wp": np.ascontiguousarray(W_proj[cs0:cs0 + CS, :]).astype(np.float32),
            "masks": mask_h,
            "ones4": np.ones((128, 4), np.float32),
        })
    return in_maps


_NC_CACHE = None


def _get_nc():
    global _NC_CACHE
    if _NC_CACHE is None:
        _NC_CACHE = build_nc()
    return _NC_CACHE


def gather(results, b_qkv, W_proj, b_proj):
    Y = np.zeros((B, T, C), np.float32)
    for i in range(NCORES):
        Y[i // HPC] += results[i]["yT"].T
    Y += (b_qkv[2 * C:].astype(np.float32) @ W_proj.astype(np.float32)
          + b_proj.astype(np.float32))[None, None, :]
    return Y


def kernel(x, W_qkv, b_qkv, W_proj, b_proj):
    global LAST_RESULT
    x = np.asarray(x, np.float32)
    W_qkv = np.asarray(W_qkv, np.float32)
    b_qkv = np.asarray(b_qkv, np.float32)
    W_proj = np.asarray(W_proj, np.float32)
    b_proj = np.asarray(b_proj, np.float32)

    nc = _get_nc()
    in_maps = make_in_maps(x, W_qkv, b_qkv, W_proj)
    res = run_bass_kernel_spmd(nc, in_maps, list(range(NCORES)), trace=TRACE)
    LAST_RESULT = res
    if TRACE and res.exec_time_ns is not None:
        print(f"HW exec time: {res.exec_time_ns} ns")
    return gather(res.results, b_qkv, W_proj, b_proj)


# revision 13
# speedup vs baseline: 1.6696x; 1.0372x over previous
"""Causal multi-head attention (B=2, T=2048, C=1024, H=16, d=64) on 8 trn2 cores.

Sharding: core i -> (batch b = i//4, head group g = i%4, 4 heads/core).
Data parallel over B, tensor parallel over heads; the out-proj partial sums
(contraction over this core's 256 channels) are reduced on the host during
the gather step, along with b_proj and the analytically-folded V bias.

Device kernel works entirely in [feature, token] (transposed) layout so no
on-device transposes are needed:
  stage 1: Q^T,K^T = (Wqk)^T x^T   (+bias, 1/sqrt(d) folded into Wq host-side)
           V       = x W_v         (natural layout, used as stage-4 lhsT)
  stage 2: S^T[j,q] = K_h^T.T @ Q_h^T   per head, causal tiles only
  stage 3: P^T = exp(S^T + mask)        (no max subtraction; scores are O(20))
  stage 4: outT[65,q] = [V_h | 1]^T.T @ P^T  accumulated over j tiles
           (row 64 = softmax denominator Z)
  stage 5: att^T = outT[0:64] * (1/Z broadcast via PE rank-1)
  stage 6: y^T = Wp.T @ att^T  -> DMA out; host sums partials + transposes.
"""

import numpy as np

import concourse.bass as bass
import concourse.mybir as mybir
from concourse import bacc
import concourse.tile as tile
from concourse.bass_utils import run_bass_kernel_spmd

B, T, C, H, D = 2, 2048, 1024, 16, 64
NCORES = 8
HPC = 4            # heads per core
CS = HPC * D       # 256 channels per core (per Q/K/V block)
KT = C // 128      # 8 contraction tiles for the projections
NT = T // 128      # 16 token tiles of 128
QB = 512           # query block (psum bank width in fp32)
NQB = T // QB      # 4 query blocks
NEG = -1e9

F32 = mybir.dt.float32
F32R = mybir.dt.float32r  # PE full rate at N>=256; matmul-input dtype

TRACE = False
LAST_RESULT = None


def _build_body(nc, tc, ctx, xT, wqk, wv, bqk, wp, masks, ones4, yT):
    AF = mybir.ActivationFunctionType

    persist = ctx.enter_context(tc.tile_pool(name="persist", bufs=1))

    wqk_sb = [persist.tile([128, 2 * CS], F32R, tag=f"wqk{k}", name=f"wqk{k}") for k in range(KT)]
    wv_sb = [persist.tile([128, CS], F32R, tag=f"wv{k}", name=f"wv{k}") for k in range(KT)]
    bqk_sb = [persist.tile([128, 1], F32, tag=f"bqk{c}", name=f"bqk{c}") for c in range(4)]
    wp_sb = [persist.tile([128, C], F32R, tag=f"wp{k}", name=f"wp{k}") for k in range(2)]
    mask_sb = persist.tile([128, 4, QB], F32, tag="mask", name="mask_sb")
    qT_sb = [persist.tile([128, T], F32R, tag=f"qT{i}", name=f"qT{i}") for i in range(2)]
    kT_sb = [persist.tile([128, T], F32R, tag=f"kT{i}", name=f"kT{i}") for i in range(2)]
    v_sb = [persist.tile([128, HPC, D + 1], F32R, tag=f"v{t}", name=f"v{t}") for t in range(NT)]
    attT_sb = [persist.tile([128, T], F32R, tag=f"attT{i}", name=f"attT{i}") for i in range(2)]

    for k in range(KT):
        nc.sync.dma_start(out=wqk_sb[k][:, :], in_=wqk[k * 128:(k + 1) * 128, :])
        nc.sync.dma_start(out=wv_sb[k][:, :], in_=wv[k * 128:(k + 1) * 128, :])
    for c4 in range(4):
        nc.sync.dma_start(out=bqk_sb[c4][:, :], in_=bqk[c4 * 128:(c4 + 1) * 128, :])
    for k in range(2):
        nc.sync.dma_start(out=wp_sb[k][:, :], in_=wp[k * 128:(k + 1) * 128, :])
    nc.sync.dma_start(out=mask_sb[:, :, :], in_=masks[:, :, :])
    for t in range(NT):
        nc.sync.dma_start(out=v_sb[t][:, :, D], in_=ones4[:, :])

    # ---------------- stage 1: projections ----------------
    with (
        tc.tile_pool(name="xpool", bufs=1) as xpool,
        tc.tile_pool(name="s1psum", bufs=3, space="PSUM") as s1p,
    ):
        xT_sb = [xpool.tile([128, T], F32R, tag=f"xT{k}", name=f"xT{k}") for k in range(KT)]
        for k in range(KT):
            nc.sync.dma_start(out=xT_sb[k][:, :], in_=xT[k * 128:(k + 1) * 128, :])

        # Q^T (c-tiles 0,1) and K^T (c-tiles 2,3): out[c, t] over k
        for ct in range(4):
            dst = qT_sb[ct] if ct < 2 else kT_sb[ct - 2]
            for tc4 in range(NQB):
                ps = s1p.tile([128, QB], F32, tag="ps", name="ps")
                for k in range(KT):
                    nc.tensor.matmul(
                        ps[:, :],
                        lhsT=wqk_sb[k][:, ct * 128:(ct + 1) * 128],
                        rhs=xT_sb[k][:, tc4 * QB:(tc4 + 1) * QB],
                        start=(k == 0),
                        stop=(k == KT - 1),
                    )
                nc.vector.tensor_scalar_add(
                    dst[:, tc4 * QB:(tc4 + 1) * QB], ps[:, :], bqk_sb[ct][:, :]
                )

        # V natural layout: V[t, c] over k; ones column appended per head
        for t in range(NT):
            ps = s1p.tile([128, QB], F32, tag="ps", name="ps")
            for k in range(KT):
                nc.tensor.matmul(
                    ps[:, 0:CS],
                    lhsT=xT_sb[k][:, t * 128:(t + 1) * 128],
                    rhs=wv_sb[k][:, :],
                    start=(k == 0),
                    stop=(k == KT - 1),
                )
            nc.any.tensor_copy(
                v_sb[t][:, :, 0:D],
                ps[:, 0:CS].rearrange("p (h d) -> p h d", h=HPC),
            )

    # ---------------- stages 2-6: attention + out-proj ----------------
    with (
        tc.tile_pool(name="sT", bufs=3, space="PSUM") as sTp,
        tc.tile_pool(name="outT", bufs=2, space="PSUM") as oTp,
        tc.tile_pool(name="pT", bufs=8) as pTp,
        tc.tile_pool(name="small", bufs=4) as smallp,
        tc.tile_pool(name="ystage", bufs=4) as ysp,
    ):
        for qb in range(NQB):
            for h in range(HPC):
                ktile = kT_sb[h // 2]
                qtile = qT_sb[h // 2]
                po = (h % 2) * D
                ngr = 2 * (qb + 1)   # groups of 2 j-tiles each
                njt = 4 * (qb + 1)
                oT = oTp.tile([D + 1, QB], F32, tag="oT", name="oT")
                for grp in range(ngr):
                    sT = sTp.tile([128, 2, QB], F32, tag="sT", name="sT")
                    for m in range(2):
                        jt = grp * 2 + m
                        nc.tensor.matmul(
                            sT[:, m, :],
                            lhsT=ktile[po:po + D, jt * 128:(jt + 1) * 128],
                            rhs=qtile[po:po + D, qb * QB:(qb + 1) * QB],
                            start=True,
                            stop=True,
                        )
                    if grp >= ngr - 2:  # diagonal groups: causal mask
                        for m in range(2):
                            jt = grp * 2 + m
                            dm = jt - 4 * qb
                            nc.vector.tensor_add(sT[:, m, :], sT[:, m, :], mask_sb[:, dm, :])
                    pT = pTp.tile([128, 2, QB], F32R, tag="pT", name="pT")
                    nc.scalar.activation(pT[:, :, :], sT[:, :, :], AF.Exp)
                    for m in range(2):
                        jt = grp * 2 + m
                        nc.tensor.matmul(
                            oT[:, :],
                            lhsT=v_sb[jt][:, h, :],
                            rhs=pT[:, m, :],
                            start=(jt == 0),
                            stop=(jt == njt - 1),
                        )
                # normalize: att^T = outT[0:D] * (1/Z), Z = outT[D]
                zrow = smallp.tile([1, QB], F32, tag="zrow", name="zrow")
                nc.scalar.copy(zrow[:, :], oT[D:D + 1, :])
                rz = smallp.tile([1, QB], F32, tag="rz", name="rz")
                nc.vector.reciprocal_approx_fast(out=rz[:, :], in_=zrow[:, :])
                zs = smallp.tile([D, QB], F32, tag="zs", name="zs")
                nc.gpsimd.partition_broadcast(zs[:, :], rz[:, :], channels=D)
                nc.vector.tensor_mul(
                    attT_sb[h // 2][po:po + D, qb * QB:(qb + 1) * QB],
                    oT[0:D, :],
                    zs[:, :],
                )

            # stage 6 for this q-block: y^T[e, qb] = Wp.T @ att^T[:, qb]
            for et in range(C // 128):
                yps_t = sTp.tile([128, 2, QB], F32, tag="sT", name="yps")
                for kc in range(2):
                    nc.tensor.matmul(
                        yps_t[:, 0, :],
                        lhsT=wp_sb[kc][:, et * 128:(et + 1) * 128],
                        rhs=attT_sb[kc][:, qb * QB:(qb + 1) * QB],
                        start=(kc == 0),
                        stop=(kc == 1),
                    )
                ys = ysp.tile([128, QB], F32, tag="ys", name="ys")
                nc.any.tensor_copy(ys[:, :], yps_t[:, 0, :])
                nc.sync.dma_start(
                    out=yT[et * 128:(et + 1) * 128, qb * QB:(qb + 1) * QB],
                    in_=ys[:, :],
                )


def build_nc():
    from contextlib import ExitStack

    nc = bacc.Bacc("TRN2", target_bir_lowering=False)
    xT = nc.dram_tensor("xT", [C, T], F32R, kind="ExternalInput")
    wqk = nc.dram_tensor("wqk", [C, 2 * CS], F32R, kind="ExternalInput")
    wv = nc.dram_tensor("wv", [C, CS], F32R, kind="ExternalInput")
    bqk = nc.dram_tensor("bqk", [2 * CS, 1], F32, kind="ExternalInput")
    wp = nc.dram_tensor("wp", [CS, C], F32R, kind="ExternalInput")
    masks = nc.dram_tensor("masks", [128, 4, QB], F32, kind="ExternalInput")
    ones4 = nc.dram_tensor("ones4", [128, 4], F32R, kind="ExternalInput")
    yT = nc.dram_tensor("yT", [C, T], F32, kind="ExternalOutput")
    with tile.TileContext(nc) as tc:
        with nc.allow_low_precision(reason="fp32r matmul inputs; accumulation stays fp32 in PSUM"):
            with ExitStack() as ctx:
                _build_body(nc, tc, ctx, xT, wqk, wv, bqk, wp, masks, ones4, yT)
    nc.compile()
    return nc


def make_masks():
    r = np.arange(128)[:, None, None]
    m = np.arange(4)[None, :, None]
    c = np.arange(QB)[None, None, :]
    return np.where(128 * m + r <= c, np.float32(0.0), np.float32(NEG)).astype(np.float32)


def make_in_maps(x, W_qkv, b_qkv, W_proj):
    scale = np.float32(1.0 / np.sqrt(D))
    mask_h = make_masks()
    in_maps = []
    for i in range(NCORES):
        b, g = divmod(i, HPC)
        cs0 = g * CS
        wq = W_qkv[:, cs0:cs0 + CS] * scale
        wk = W_qkv[:, C + cs0:C + cs0 + CS]
        bq = b_qkv[cs0:cs0 + CS] * scale
        bk = b_qkv[C + cs0:C + cs0 + CS]
        in_maps.append({
            "xT": np.ascontiguousarray(x[b].T).astype(np.float32),
            "wqk": np.concatenate([wq, wk], axis=1).astype(np.float32),
            "wv": np.ascontiguousarray(W_qkv[:, 2 * C + cs0:2 * C + cs0 + CS]).astype(np.float32),
            "bqk": np.concatenate([bq, bk])[:, None].astype(np.float32),
            "wp": np.ascontiguousarray(W_proj[cs0:cs0 + CS, :]).astype(np.float32),
            "masks": mask_h,
            "ones4": np.ones((128, 4), np.float32),
        })
    return in_maps


_NC_CACHE = None


def _get_nc():
    global _NC_CACHE
    if _NC_CACHE is None:
        _NC_CACHE = build_nc()
    return _NC_CACHE


def gather(results, b_qkv, W_proj, b_proj):
    Y = np.zeros((B, T, C), np.float32)
    for i in range(NCORES):
        Y[i // HPC] += results[i]["yT"].T
    Y += (b_qkv[2 * C:].astype(np.float32) @ W_proj.astype(np.float32)
          + b_proj.astype(np.float32))[None, None, :]
    return Y


def kernel(x, W_qkv, b_qkv, W_proj, b_proj):
    global LAST_RESULT
    x = np.asarray(x, np.float32)
    W_qkv = np.asarray(W_qkv, np.float32)
    b_qkv = np.asarray(b_qkv, np.float32)
    W_proj = np.asarray(W_proj, np.float32)
    b_proj = np.asarray(b_proj, np.float32)

    nc = _get_nc()
    in_maps = make_in_maps(x, W_qkv, b_qkv, W_proj)
    res = run_bass_kernel_spmd(nc, in_maps, list(range(NCORES)), trace=TRACE)
    LAST_RESULT = res
    if TRACE and res.exec_time_ns is not None:
        print(f"HW exec time: {res.exec_time_ns} ns")
    return gather(res.results, b_qkv, W_proj, b_proj)
